# revision 4
# baseline (speedup 1.0000x reference)
"""BEVFeatureExtractorV2 Trainium2 kernel.

Computes, for each ROI box, 5 sample points (center + 4 edge midpoints of the
rotated box) and bilinearly interpolates a [C,H,W] BEV feature map at those
points, producing [B, N, 5*C].

Sharding: 8 cores = 4 batches x 2 halves of the 512 rois. Each core receives
its batch's feature map (re-laid-out on host) and 256 rois.

Device strategy (per core):
  - Host pre-lays the feature map as table2[y*W+x] = [im[y,x,:], im[y+1,x,:]]
    (shape [H*W, 2C]) so ONE indirect-DMA descriptor (4KB) fetches all 4
    bilinear neighbors of a point: entries e and e+1 give pixels
    (y0,x0),(y0+1,x0),(y0,x0+1),(y0+1,x0+1).
  - On device: compute the 5 points per roi (sin on ACT engine), directly
    in pixel space pre-shifted by -0.5 so the HW round-to-nearest f32->i32
    convert yields floor(); gather indices in int arithmetic; 4 bilinear
    weights; then for each of 10 tiles of 128 points: indirect-gather
    [128, 4C], weight chunks 0-1 on DVE (broadcast-AP multiply) and chunks
    2-3 on ACT (Copy with per-partition scale), fold with 2 adds, store
    [128, C] to the output slab. Measured ~35.3-35.9 us per core (min/median) end-to-end
    (bench.py; ~6.5 MB DMA/core, HBM roofline ~18 us).
"""

import os
import numpy as np

import concourse.bass as bass
import concourse.bacc as bacc
import concourse.tile as tile
from concourse import mybir
from concourse.bass_utils import run_bass_kernel_spmd

F32 = mybir.dt.float32
I32 = mybir.dt.int32

B, N, C, H, W = 4, 512, 256, 256, 256
NCORES = 8
NR = N * B // NCORES          # rois per core = 256
NPT = 5                       # sample points per roi
P = 128                       # partitions
NT = NR // P                  # roi tiles per core = 2
PC_START = -51.2
INV_VOX = 2.5                 # 1 / (0.1 * 4)
BIAS = -PC_START * INV_VOX    # 128.0

_CACHED = {}


def build_program(repeat=1, loop_iters=None, bufs=(10, 4, 4, 6), gg=1,
                  variant="full", nswq=1):
    """variant: full | nostore | gather | gatherNk (N KB per point) |
    compute (single gather, full compute+store)."""
    import contextlib
    do_compute = variant in ("full", "nostore", "compute")
    do_store = variant in ("full", "compute")
    gbytes = 4096
    if variant.startswith("gather") and variant != "gather":
        gbytes = int(variant[6:-1]) * 1024
    nc = bacc.Bacc("TRN2", target_bir_lowering=False, debug=False,
                   enable_asserts=False, num_swdge_queues=nswq)
    table = nc.dram_tensor("table2", [H * W, 2 * C], F32, kind="ExternalInput").ap()
    rois = nc.dram_tensor("rois", [NR, 7], F32, kind="ExternalInput").ap()
    out = nc.dram_tensor("out", [NR, NPT * C], F32, kind="ExternalOutput").ap()

    NJ = NT * NPT  # 10 gather tiles
    with tile.TileContext(nc) as tc:
        with tc.tile_pool(name="coord", bufs=min(2, repeat)) as cp, \
             tc.tile_pool(name="gather", bufs=bufs[0]) as gp, \
             tc.tile_pool(name="mul", bufs=bufs[1]) as mp, \
             tc.tile_pool(name="fold", bufs=bufs[2]) as sp, \
             tc.tile_pool(name="outp", bufs=bufs[3]) as op, \
             (tc.For_i(0, loop_iters, 1) if loop_iters
              else contextlib.nullcontext()):
          for _rep in range(repeat):
            # ---- load rois: [256,7] -> [128, (t d)] -------------------
            R = cp.tile([P, NT * 7], F32)
            R3 = R[:].rearrange("p (t d) -> p t d", t=NT)
            nc.gpsimd.dma_start(R3, rois.rearrange("(t p) d -> p t d", t=NT))

            cx = R3[:, :, 0]
            cy = R3[:, :, 1]
            ry = R3[:, :, 6]

            # ---- trig (ACT) -----------------------------------------
            zero = cp.tile([P, 1], F32)
            halfpi = cp.tile([P, 1], F32)
            nc.vector.memset(zero[:], 0.0)
            nc.vector.memset(halfpi[:], float(np.pi / 2))
            trig = cp.tile([P, 2 * NT], F32)
            t3 = trig[:].rearrange("p (a t) -> p a t", a=2)
            sn, cs = t3[:, 0, :], t3[:, 1, :]
            nc.scalar.activation(sn, ry, mybir.ActivationFunctionType.Sin,
                                 bias=zero[:])
            # cos(x) = sin(pi/2 - |x|), argument stays within [-pi/2, pi/2]
            ab = cp.tile([P, NT], F32)
            nc.scalar.activation(ab[:], ry, mybir.ActivationFunctionType.Abs,
                                 bias=zero[:])
            nc.scalar.activation(cs, ab[:], mybir.ActivationFunctionType.Sin,
                                 bias=halfpi[:], scale=-1.0)

            # ---- pixel-space center (ACT), pre-shifted by -0.5 ------
            # HW f32->i32 convert is round-to-nearest, so
            # convert(xs - 0.5) == floor(xs) (ties land on a value-correct
            # neighbor; frac is recomputed off the chosen neighbor below).
            ctr = cp.tile([P, 2 * NT], F32)
            c3 = ctr[:].rearrange("p (a t) -> p a t", a=2)
            xc, yc = c3[:, 0, :], c3[:, 1, :]
            nc.scalar.activation(xc, cx, mybir.ActivationFunctionType.Copy,
                                 bias=BIAS - 0.5, scale=INV_VOX)
            nc.scalar.activation(yc, cy, mybir.ActivationFunctionType.Copy,
                                 bias=BIAS - 0.5, scale=INV_VOX)

            # ---- scaled half-dims (GPSIMD, idle here); 1.25 = 0.5*2.5
            hd = cp.tile([P, 2 * NT], F32)
            h3 = hd[:].rearrange("p (a t) -> p a t", a=2)
            hx, hy = h3[:, 0, :], h3[:, 1, :]
            nc.gpsimd.tensor_scalar_mul(hx, R3[:, :, 3], 1.25)
            nc.gpsimd.tensor_scalar_mul(hy, R3[:, :, 4], 1.25)

            # ---- rotated pixel offsets: x-pair DVE, y-pair GPSIMD ---
            rot = cp.tile([P, 4 * NT], F32)
            r3 = rot[:].rearrange("p (a t) -> p a t", a=4)
            rxc, rxs, rys, ryc = (r3[:, a, :] for a in range(4))
            nc.vector.tensor_mul(rxc, hx, cs)
            nc.vector.tensor_mul(rys, hy, sn)
            nc.gpsimd.tensor_mul(rxs, hx, sn)
            nc.gpsimd.tensor_mul(ryc, hy, cs)

            # ---- 5 points per roi in pixel space: x on DVE, y GPSIMD
            XY = cp.tile([P, 2 * NJ], F32)
            x4 = XY[:].rearrange("p (a t k) -> p a t k", a=2, t=NT)
            xs3, ys3 = x4[:, 0, :, :], x4[:, 1, :, :]
            nc.vector.tensor_copy(xs3[:, :, 0], xc)
            nc.vector.tensor_sub(xs3[:, :, 1], xc, rxc)   # front
            nc.vector.tensor_add(xs3[:, :, 2], xc, rxc)   # back
            nc.vector.tensor_sub(xs3[:, :, 3], xc, rys)   # left
            nc.vector.tensor_add(xs3[:, :, 4], xc, rys)   # right
            nc.gpsimd.tensor_copy(ys3[:, :, 0], yc)
            nc.gpsimd.tensor_add(ys3[:, :, 1], yc, rxs)
            nc.gpsimd.tensor_sub(ys3[:, :, 2], yc, rxs)
            nc.gpsimd.tensor_sub(ys3[:, :, 3], yc, ryc)
            nc.gpsimd.tensor_add(ys3[:, :, 4], yc, ryc)

            # ---- floor via RNE convert of pre-shifted coords --------
            XYi = cp.tile([P, 2 * NJ], I32)
            nc.vector.tensor_copy(XYi[:], XY[:])   # = floor(true coords)

            # ---- gather index in int arithmetic (gathers launch early)
            idx = cp.tile([P, NJ], I32)
            nc.vector.tensor_scalar(idx[:], XYi[:, NJ:], W, None,
                                    mybir.AluOpType.mult)
            nc.vector.tensor_add(idx[:], idx[:], XYi[:, :NJ])

            # ---- fracs + complements (overlap the gathers) ----------
            XYf = cp.tile([P, 2 * NJ], F32)
            D = cp.tile([P, 2 * NJ], F32)
            XYr = cp.tile([P, 2 * NJ], F32)
            XYg = cp.tile([P, 2 * NJ], F32)
            nc.vector.tensor_copy(XYf[:], XYi[:])
            nc.vector.tensor_sub(D[:], XY[:], XYf[:])        # frac - 0.5
            nc.vector.tensor_scalar(XYr[:], D[:], 0.5, None,
                                    mybir.AluOpType.add)     # frac
            nc.vector.tensor_scalar(XYg[:], D[:], -1.0, 0.5,
                                    mybir.AluOpType.mult, mybir.AluOpType.add)
            fx, fy = XYr[:, :NJ], XYr[:, NJ:]
            gx, gy = XYg[:, :NJ], XYg[:, NJ:]
            Wt = cp.tile([P, 4 * NJ], F32)
            W3 = Wt[:].rearrange("p (j w) -> p j w", w=4)
            nc.vector.tensor_mul(W3[:, :, 0], gx, gy)
            nc.vector.tensor_mul(W3[:, :, 1], gx, fy)
            nc.vector.tensor_mul(W3[:, :, 2], fx, gy)
            nc.vector.tensor_mul(W3[:, :, 3], fx, fy)

            # ---- gather + weighted fold per (t, k) tile -------------
            # gg points gathered per indirect DMA (2D dest, flat layout)
            gelem = gbytes // 4
            Gprev = None
            for t in range(NT):
                Gb = None
                for k in range(NPT):
                    j = t * NPT + k
                    if variant == "compute" and Gprev is not None:
                        Gb = Gprev
                    elif k % gg == 0:
                        ng = min(gg, NPT - k)
                        Gb = gp.tile([P, ng * gelem], F32, tag="G")
                        nc.gpsimd.indirect_dma_start(
                            out=Gb[:],
                            out_offset=None,
                            in_=table,
                            in_offset=bass.IndirectOffsetOnAxis(
                                ap=idx[:, j:j + ng], axis=0),
                        )
                        Gprev = Gb
                    if not do_compute:
                        continue
                    G = Gb[:, (k % gg) * 4 * C:(k % gg + 1) * 4 * C]
                    # chunks 0,1 weighted on DVE; chunks 2,3 on ACT (idle)
                    M = mp.tile([P, 4 * C], F32, tag="M")
                    nc.vector.tensor_mul(
                        M[:, :2 * C].rearrange("p (a c) -> p a c", a=2),
                        G[:, :2 * C].rearrange("p (a c) -> p a c", a=2),
                        W3[:, j, 0:2].unsqueeze(2).to_broadcast([P, 2, C]),
                    )
                    nc.scalar.activation(
                        M[:, 2 * C:3 * C], G[:, 2 * C:3 * C],
                        mybir.ActivationFunctionType.Copy,
                        bias=0.0, scale=W3[:, j, 2:3])
                    nc.scalar.activation(
                        M[:, 3 * C:4 * C], G[:, 3 * C:4 * C],
                        mybir.ActivationFunctionType.Copy,
                        bias=0.0, scale=W3[:, j, 3:4])
                    S = sp.tile([P, 2 * C], F32, tag="S")
                    nc.vector.tensor_add(S[:], M[:, :2 * C], M[:, 2 * C:])
                    O = op.tile([P, C], F32, tag="O")
                    nc.vector.tensor_add(O[:], S[:, :C], S[:, C:])
                    if do_store:
                        nc.sync.dma_start(
                            out[t * P:(t + 1) * P, k * C:(k + 1) * C], O[:])
    nc.compile()
    return nc


def _get_program():
    if "nc" not in _CACHED:
        _CACHED["nc"] = build_program()
    return _CACHED["nc"]


def _make_table2(feats):
    """feats: [B,C,H,W] f32 -> list of B arrays [H*W, 2C] (channel-last,
    row y and y+1 concatenated)."""
    tables = []
    for b in range(B):
        bev = np.ascontiguousarray(feats[b].transpose(1, 2, 0))  # [H,W,C]
        nxt = bev[np.minimum(np.arange(H) + 1, H - 1)]           # [H,W,C]
        t2 = np.concatenate([bev, nxt], axis=2)                  # [H,W,2C]
        tables.append(np.ascontiguousarray(t2.reshape(H * W, 2 * C)))
    return tables


def kernel(spatial_features_2d, rois, _want_results=False):
    feats = np.asarray(spatial_features_2d, dtype=np.float32)
    rois_np = np.asarray(rois, dtype=np.float32)
    assert feats.shape == (B, C, H, W) and rois_np.shape == (B, N, 7)

    nc = _get_program()
    tables = _make_table2(feats)
    in_maps = []
    for core in range(NCORES):
        b, h = divmod(core, 2)
        in_maps.append({
            "table2": tables[b],
            "rois": np.ascontiguousarray(rois_np[b, h * NR:(h + 1) * NR]),
        })

    try:
        res = run_bass_kernel_spmd(
            nc, in_maps, list(range(NCORES)),
            trace=bool(int(os.environ.get("BEV_TRACE", "0"))),
        )
    except ModuleNotFoundError:
        # BASS_TRACE in the environment routes through the NTFF profile
        # hook (antenv.axon_hooks), which some containers lack. Degrade to
        # an untraced run instead of failing.
        os.environ["BASS_NEVER_TRACE"] = "1"
        try:
            res = run_bass_kernel_spmd(nc, in_maps, list(range(NCORES)),
                                       trace=False)
        finally:
            os.environ.pop("BASS_NEVER_TRACE", None)

    out = np.empty((B, N, NPT * C), dtype=np.float32)
    for core in range(NCORES):
        b, h = divmod(core, 2)
        out[b, h * NR:(h + 1) * NR] = res.results[core]["out"]
    if _want_results:
        return out, res
    return out

